# revision 15
# baseline (speedup 1.0000x reference)
"""CrossATT kernel for Trainium2 (Bass/Tile), data-parallel over batch on 8 cores.

Math (per batch b):
    S = x_cont @ x_ques^T            # [C, Q]
    A = softmax(S, axis=-1)          # over q
    c2q = A @ x_ques                 # [C, D]
    out = c2q @ W1 + x_cont @ W0     # [C, D]

Device-side formulation works fully transposed so the TensorE contraction
axis is always on partitions and softmax needs no on-chip transposes.
W1 is folded into x_ques on the host (QW = x_ques @ W1) and the W0 term
(x_cont @ W0, no attention dependence) is added on the host, so the device
computes only:
    ST[q, c]  = sum_d QT[d, q] * XT[d, c]         (MM1, per 128-q chunk)
    E         = exp(ST)                           (no max subtraction: |S| < ~70)
    s[c]      = sum_q E[q, c]                     (partial adds + ones-matmul)
    OT[e, c]  = (sum_q QW[q, e] * E[q, c]) / s[c] (MM2 + reciprocal broadcast mul)
host:
    out = OT^T + x_cont @ W0

MM1 runs in float32r (TF32-class, 1 cycle/row at moving width >= 256); E, QW
and the OT output are bf16 (same 1 cycle/row on the PE, but 2x DVE throughput
for the partial adds and half the OT store traffic; measured end-to-end error
~4e-3 vs the 2e-2 gate). The softmax denominator reciprocal uses the
single-instruction reciprocal_approx_fast (~18 bits) instead of the stock
8-instruction reciprocal, and the row broadcast runs on the otherwise-idle
GPSIMD engine.
"""

import os

import numpy as np

import concourse.bass as bass
import concourse.bass_isa as bass_isa
import concourse.mybir as mybir
import concourse.tile as tile
from concourse import bacc, library_config
from concourse.bass_utils import run_bass_kernel_spmd

B, C_LEN, Q_LEN, D = 16, 4096, 512, 128
NCORES = 8
BPC = B // NCORES          # batches per core
CB = 512                   # c-block width (PSUM bank / max f32 moving width)
NBLK = C_LEN // CB         # 8 blocks per batch
NQ = Q_LEN // 128          # 4 q-chunks

F32 = mybir.dt.float32
F16 = mybir.dt.float16
BF16 = mybir.dt.bfloat16

_CACHE = {}


def _build():
    nc = bacc.Bacc("TRN2", target_bir_lowering=False, debug=False, num_devices=NCORES)

    XT = nc.declare_dram_parameter("XT", [BPC, D, C_LEN], F16, isOutput=False)
    QT = nc.declare_dram_parameter("QT", [BPC, D, Q_LEN], F16, isOutput=False)
    QW = nc.declare_dram_parameter("QW", [BPC, 128, NQ, D], BF16, isOutput=False)
    OT = nc.declare_dram_parameter("OT", [BPC, D, C_LEN], BF16, isOutput=True)

    # Timing-only knob: KREPEAT>1 repeats the computation inside one NEFF
    # with per-repeat checksum outputs keeping every repeat alive.
    repeat = int(os.environ.get("KREPEAT", "1"))
    CS = None
    if repeat > 1:
        CS = nc.declare_dram_parameter("CS", [repeat * BPC, NBLK], F32, isOutput=True)

    with tile.TileContext(nc) as tc:
        with (
            tc.tile_pool(name="const", bufs=1) as const,
            tc.tile_pool(name="xt", bufs=1) as xtp,
            tc.tile_pool(name="e", bufs=6) as ep,
            tc.tile_pool(name="padd", bufs=3) as paddp,
            tc.tile_pool(name="r", bufs=3) as rp,
            tc.tile_pool(name="rbc", bufs=3) as rbcp,
            tc.tile_pool(name="osb", bufs=3) as osbp,
            tc.tile_pool(name="ps_st", bufs=2, space="PSUM") as ps_st,
            tc.tile_pool(name="ps_s", bufs=1, space="PSUM") as ps_s,
            tc.tile_pool(name="ps_o", bufs=3, space="PSUM") as ps_o,
        ):
            nc.gpsimd.load_library(library_config.proxy)

            # column of 128 ones: stationary for the s-sum matmul
            ones_f = const.tile([128, 1], F32)
            nc.vector.memset(ones_f, 1.0)
            ones_b = const.tile([128, 1], BF16)
            nc.vector.tensor_copy(out=ones_b, in_=ones_f)

            # Preamble DMA order is latency-critical: qt0 gates the first
            # LDWEIGHTS and xt pair 0 the first matmul, so qt0 leads the
            # scalar DGE ring while xt00 leads the sync ring; everything
            # else (qt1, qw, remaining xt pairs) queues behind them. All
            # XT loads are issued upfront with per-pair semaphores so
            # steady-state MM1 never waits on the ring (~32KB/partition).
            def xt_load(b, jp):
                t = xtp.tile([D, 2, CB], F16, tag=f"xt{b}_{jp}")
                nc.sync.dma_start(
                    out=t,
                    in_=XT[b][:, bass.ts(jp, 2 * CB)].rearrange(
                        "d (u c) -> d u c", u=2
                    ),
                )
                return t

            qt_sb = []
            for b in range(BPC):
                qt = const.tile([D, Q_LEN], F16, name=f"qt{b}")
                nc.scalar.dma_start(out=qt, in_=QT[b])
                qt_sb.append(qt)
            xt_sb = {}
            xt_sb[(0, 0)] = xt_load(0, 0)
            qw_sb = []
            for b in range(BPC):
                qw = const.tile([128, NQ, D], BF16, name=f"qw{b}")
                nc.scalar.dma_start(out=qw, in_=QW[b])
                qw_sb.append(qw)
            for b in range(BPC):
                for jp in range(NBLK // 2):
                    if (b, jp) not in xt_sb:
                        xt_sb[(b, jp)] = xt_load(b, jp)

            # Three-stage software pipeline over the flat block sequence:
            #   stage1(j):   XT DMA (per pair), MM1, exp, partial adds
            #   stage2(j-1): MM2, ones-matmul (s), reciprocal, broadcast
            #   stage3(j-2): normalize-mul, OT store
            # Every tensor op's operands are ready a full block before it
            # issues, so the in-order tensor queue never stalls on the
            # exp -> adds -> s chain of the same block.
            blocks = []
            for b_rep in range(repeat * BPC):
                for jp in range(NBLK // 2):
                    for u in range(2):
                        blocks.append((b_rep, jp, u))
            st1 = {}  # j -> (e_pairs, part, o_pair, blk)
            st2 = {}  # j -> (o_ps, r_bc, o_pair, blk)

            def stage1(j):
                b_rep, jp, u = blocks[j]
                b = b_rep % BPC
                if u == 0:
                    o_pair = osbp.tile([D, 2, CB], BF16, tag="osb")
                    st1["o_pair"] = o_pair
                xt_blk = xt_sb[(b, jp)][:, u, :]

                # MM1 into paired PSUM tiles; one exp per pair (halves the
                # ACTIVATE fixed overhead), E in bf16
                e_pairs = []
                for h in range(NQ // 2):
                    st = ps_st.tile([128, 2, CB], F32, tag="st")
                    for i in range(2):
                        k = 2 * h + i
                        nc.tensor.matmul(
                            out=st[:, i, :],
                            lhsT=qt_sb[b][:, bass.ts(k, 128)],
                            rhs=xt_blk,
                            start=True,
                            stop=True,
                        )
                    e = ep.tile([128, 2, CB], BF16, tag="e")
                    nc.scalar.activation(
                        out=e, in_=st, func=mybir.ActivationFunctionType.Exp
                    )
                    e_pairs.append(e)

                # partial sums over q chunks (bf16, 2x DVE rate): one
                # full-pair [128,1024] add, then a 512-wide fold
                with nc.allow_low_precision(
                    reason="bf16 partial softmax sums; denominator only "
                    "needs ~3 digits (gate is 2e-2)"
                ):
                    q0 = paddp.tile([128, 2, CB], BF16, tag="q0")
                    nc.vector.tensor_add(
                        out=q0, in0=e_pairs[0], in1=e_pairs[1]
                    )
                    part = paddp.tile([128, CB], BF16, tag="part")
                    nc.vector.tensor_add(
                        out=part, in0=q0[:, 0, :], in1=q0[:, 1, :]
                    )
                st1[j] = (e_pairs, part, st1["o_pair"], (b_rep, jp, u))

            def s_chain(j):
                # s for block j-2: part is two iterations old, so the
                # ones-matmul (first on this iteration's tensor queue)
                # never waits; recip/broadcast complete with a full
                # iteration of slack before the mul consumes r_bc.
                part = st1[j][1]
                s_ps = ps_s.tile([1, CB], F32)
                nc.tensor.matmul(
                    out=s_ps, lhsT=ones_b, rhs=part, start=True, stop=True
                )
                # single-instruction approximate reciprocal (~18 bits)
                r_sb = rp.tile([1, CB], F32)
                nc.vector.reciprocal_approx_fast(out=r_sb, in_=s_ps)
                # broadcast recip row across partitions on GPSIMD
                r_bc = rbcp.tile([128, CB], F32)
                nc.gpsimd.partition_broadcast(r_bc, r_sb)
                st2[j] = r_bc

            def mm2(j):
                e_pairs, part, o_pair, (b_rep, jp, u) = st1[j]
                b = b_rep % BPC
                # MM2: OT1 = QW^T E (unnormalized c2q@W1, transposed)
                o_ps = ps_o.tile([D, CB], F32)
                for h in range(NQ // 2):
                    for i in range(2):
                        k = 2 * h + i
                        nc.tensor.matmul(
                            out=o_ps,
                            lhsT=qw_sb[b][:, k, :],
                            rhs=e_pairs[h][:, i, :],
                            start=(k == 0),
                            stop=(k == NQ - 1),
                        )
                st1[j] = (e_pairs, part, o_pair, (b_rep, jp, u), o_ps)

            def mul_store(j):
                _, _, o_pair, (b_rep, jp, u), o_ps = st1.pop(j)
                r_bc = st2.pop(j)
                # normalize while moving PSUM->SBUF (bf16 out)
                with nc.allow_low_precision(
                    reason="bf16 OT output; 2e-2 gate, measured ~4e-3"
                ):
                    nc.vector.tensor_mul(out=o_pair[:, u, :], in0=o_ps, in1=r_bc)
                if CS is not None:
                    nc.sync.dma_start(
                        out=CS[b_rep : b_rep + 1, 2 * jp + u : 2 * jp + u + 1],
                        in_=o_pair[0:1, u, 0:1],
                    )
                last_pair = jp == NBLK // 2 - 1 and b_rep == repeat * BPC - 1
                if repeat == 1 or b_rep >= (repeat - 1) * BPC:
                    if last_pair:
                        # final pair: store per block so the last DMA
                        # starts right after its mul instead of waiting
                        # for both muls
                        nc.sync.dma_start(
                            out=OT[b_rep % BPC][
                                :, bass.ts(2 * jp + u, CB)
                            ],
                            in_=o_pair[:, u, :],
                        )
                    elif u == 1:
                        nc.sync.dma_start(
                            out=OT[b_rep % BPC][
                                :, bass.ts(jp, 2 * CB)
                            ].rearrange("d (u c) -> d u c", u=2),
                            in_=o_pair,
                        )

            n = len(blocks)
            for j in range(n + 2):
                if j >= 2:
                    s_chain(j - 2)
                if j < n:
                    stage1(j)
                if j >= 1 and j - 1 < n:
                    mm2(j - 1)
                if j >= 2:
                    mul_store(j - 2)

    nc.compile()
    return nc


def _prep_inputs(x_cont, x_ques, W1):
    bf16 = mybir.dt.np(BF16)
    xt = np.ascontiguousarray(
        x_cont.transpose(0, 2, 1), dtype=np.float16
    )                                                          # [B, D, C] f16
    qt = np.ascontiguousarray(
        x_ques.transpose(0, 2, 1), dtype=np.float16
    )                                                          # [B, D, Q] f16
    qw = np.matmul(x_ques, W1).astype(bf16)                    # [B, Q, D] bf16
    # device-side stationary layout: [B, 128, NQ, D], contiguous DMA lines
    qw = np.ascontiguousarray(
        qw.reshape(B, Q_LEN // 128, 128, D).transpose(0, 2, 1, 3)
    )
    return xt, qt, qw


def kernel(x_cont, x_ques, c_mask, q_mask, W0, W1):
    x_cont = np.ascontiguousarray(x_cont, dtype=np.float32)
    x_ques = np.ascontiguousarray(x_ques, dtype=np.float32)
    W0 = np.ascontiguousarray(W0, dtype=np.float32)
    W1 = np.ascontiguousarray(W1, dtype=np.float32)

    if "nc" not in _CACHE:
        _CACHE["nc"] = _build()
    nc = _CACHE["nc"]

    xt, qt, qw = _prep_inputs(x_cont, x_ques, W1)

    in_maps = []
    for i in range(NCORES):
        sl = slice(i * BPC, (i + 1) * BPC)
        in_maps.append({"XT": xt[sl], "QT": qt[sl], "QW": qw[sl]})

    res = run_bass_kernel_spmd(nc, in_maps, core_ids=list(range(NCORES)))

    out = np.matmul(x_cont, W0)  # [B, C, D] — attention-free term, on host
    for i in range(NCORES):
        ot = np.asarray(res.results[i]["OT"]).astype(np.float32)  # [BPC, D, C]
        out[i * BPC : (i + 1) * BPC] += ot.transpose(0, 2, 1)
    return out


# --- timing helper for test.py (not used by the graded kernel() path) ---
def timed_run(x_cont, x_ques, W0, W1, iters=10):
    """Persistent-jit execution; returns (list of wall times per exec, out).

    Replicates bass2jax.run_bass_via_pjrt but keeps the jitted callable and
    device-resident inputs across iterations so the measured time is
    dispatch + NEFF execution, not retracing/host transfers.
    """
    import time

    import jax
    from jax.sharding import Mesh, PartitionSpec
    from jax.experimental.shard_map import shard_map

    import concourse.mybir as _mybir
    from concourse import bass2jax

    if "nc" not in _CACHE:
        _CACHE["nc"] = _build()
    nc = _CACHE["nc"]
    bass2jax.install_neuronx_cc_hook()

    xt, qt, qw = _prep_inputs(x_cont, x_ques, W1)
    full = {"XT": xt, "QT": qt, "QW": qw}

    partition_name = nc.partition_id_tensor.name if nc.partition_id_tensor else None
    in_names, out_names, out_avals, zero_outs = [], [], [], []
    for alloc in nc.m.functions[0].allocations:
        if not isinstance(alloc, _mybir.MemoryLocationSet):
            continue
        name = alloc.memorylocations[0].name
        if alloc.kind == "ExternalInput":
            if name != partition_name:
                in_names.append(name)
        elif alloc.kind == "ExternalOutput":
            shape = tuple(alloc.tensor_shape)
            dtype = _mybir.dt.np(alloc.dtype)
            out_names.append(name)
            out_avals.append(jax.core.ShapedArray(shape, dtype))
            zero_outs.append(np.zeros(shape, dtype))
    n_params = len(in_names)
    n_outs = len(out_avals)
    all_names = in_names + out_names
    if partition_name is not None:
        all_names = all_names + [partition_name]

    def _body(*args):
        operands = list(args)
        if partition_name is not None:
            operands.append(bass2jax.partition_id_tensor())
        outs = bass2jax._bass_exec_p.bind(
            *operands,
            out_avals=tuple(out_avals),
            in_names=tuple(all_names),
            out_names=tuple(out_names),
            lowering_input_output_aliases=(),
            sim_require_finite=True,
            sim_require_nnan=True,
            nc=nc,
        )
        return tuple(outs)

    devices = jax.devices()[:NCORES]
    mesh = Mesh(np.asarray(devices), ("core",))
    spec = PartitionSpec("core")
    donate = tuple(range(n_params, n_params + n_outs))
    sharded = jax.jit(
        shard_map(
            _body,
            mesh=mesh,
            in_specs=(spec,) * (n_params + n_outs),
            out_specs=(spec,) * n_outs,
            check_rep=False,
        ),
        donate_argnums=donate,
        keep_unused=True,
    )

    sharding = jax.sharding.NamedSharding(mesh, spec)
    concat_in = [
        jax.device_put(np.ascontiguousarray(full[name]), sharding)
        for name in in_names
    ]

    def fresh_zeros():
        return [
            jax.device_put(
                np.zeros((NCORES * z.shape[0], *z.shape[1:]), z.dtype), sharding
            )
            for z in zero_outs
        ]

    out_arrs = sharded(*concat_in, *fresh_zeros())
    jax.block_until_ready(out_arrs)

    zsets = [fresh_zeros() for _ in range(iters)]
    times = []
    for zs in zsets:
        t0 = time.perf_counter()
        out_arrs = sharded(*concat_in, *zs)
        jax.block_until_ready(out_arrs)
        times.append(time.perf_counter() - t0)
    return times, out_arrs


# revision 17
# speedup vs baseline: 1.0299x; 1.0299x over previous
"""CrossATT kernel for Trainium2 (Bass/Tile), data-parallel over batch on 8 cores.

Math (per batch b):
    S = x_cont @ x_ques^T            # [C, Q]
    A = softmax(S, axis=-1)          # over q
    c2q = A @ x_ques                 # [C, D]
    out = c2q @ W1 + x_cont @ W0     # [C, D]

Device-side formulation works fully transposed so the TensorE contraction
axis is always on partitions and softmax needs no on-chip transposes.
W1 is folded into x_ques on the host (QW = x_ques @ W1) and the W0 term
(x_cont @ W0, no attention dependence) is added on the host, so the device
computes only:
    ST[q, c]  = sum_d QT[d, q] * XT[d, c]         (MM1, per 128-q chunk)
    E         = exp(ST)                           (no max subtraction: |S| < ~70)
    s[c]      = sum_q E[q, c]                     (partial adds + ones-matmul)
    OT[e, c]  = (sum_q QW[q, e] * E[q, c]) / s[c] (MM2 + reciprocal broadcast mul)
host:
    out = OT^T + x_cont @ W0

MM1 inputs are fp16 (same 10-bit mantissa as float32r for these unit-variance
inputs, but 2-byte PE streaming: measured 379 ns vs 501 ns per 512-row
matmul); E, QW and the OT output are bf16 (exp range needs 8 exponent bits;
2x DVE throughput on the partial adds; half the OT store traffic). Measured
end-to-end error ~3.4e-3 vs the 2e-2 gate. The softmax denominator
reciprocal is the single-instruction reciprocal_approx_fast (~18 bits)
instead of the stock 8-instruction reciprocal; the row broadcast runs on the
otherwise-idle GPSIMD engine.

The emission is a three-stage software pipeline (MM1/exp/adds for block j,
MM2 + s-chain for j-1, normalize/store for j-2) so the in-order tensor queue
never waits on the exp -> adds -> s chain: the steady-state matmul stream
runs gapless at the PE's full clock (~2.6 us per 512-column block). All XT
loads are issued upfront with per-pair semaphores; qw/qt ride the scalar
engine's DGE ring in parallel with the sync ring.
"""

import os

import numpy as np

import concourse.bass as bass
import concourse.bass_isa as bass_isa
import concourse.mybir as mybir
import concourse.tile as tile
from concourse import bacc, library_config
from concourse.bass_utils import run_bass_kernel_spmd

B, C_LEN, Q_LEN, D = 16, 4096, 512, 128
NCORES = 8
BPC = B // NCORES          # batches per core
CB = 512                   # c-block width (PSUM bank / max f32 moving width)
NBLK = C_LEN // CB         # 8 blocks per batch
NQ = Q_LEN // 128          # 4 q-chunks

F32 = mybir.dt.float32
F16 = mybir.dt.float16
BF16 = mybir.dt.bfloat16

_CACHE = {}


def _build():
    nc = bacc.Bacc("TRN2", target_bir_lowering=False, debug=False, num_devices=NCORES)

    XT = nc.declare_dram_parameter("XT", [BPC, D, C_LEN], F16, isOutput=False)
    QT = nc.declare_dram_parameter("QT", [BPC, D, Q_LEN], F16, isOutput=False)
    QW = nc.declare_dram_parameter("QW", [BPC, 128, NQ, D], BF16, isOutput=False)
    OT = nc.declare_dram_parameter("OT", [BPC, D, C_LEN], BF16, isOutput=True)

    # Timing-only knob: KREPEAT>1 repeats the computation inside one NEFF
    # with per-repeat checksum outputs keeping every repeat alive.
    repeat = int(os.environ.get("KREPEAT", "1"))
    CS = None
    if repeat > 1:
        CS = nc.declare_dram_parameter("CS", [repeat * BPC, NBLK], F32, isOutput=True)

    with tile.TileContext(nc) as tc:
        with (
            tc.tile_pool(name="const", bufs=1) as const,
            tc.tile_pool(name="xt", bufs=1) as xtp,
            tc.tile_pool(name="e", bufs=8) as ep,
            tc.tile_pool(name="padd", bufs=4) as paddp,
            tc.tile_pool(name="r", bufs=3) as rp,
            tc.tile_pool(name="rbc", bufs=4) as rbcp,
            tc.tile_pool(name="osb", bufs=3) as osbp,
            tc.tile_pool(name="ps_st", bufs=2, space="PSUM") as ps_st,
            tc.tile_pool(name="ps_s", bufs=1, space="PSUM") as ps_s,
            tc.tile_pool(name="ps_o", bufs=3, space="PSUM") as ps_o,
        ):
            nc.gpsimd.load_library(library_config.proxy)

            # column of 128 ones: stationary for the s-sum matmul
            ones_f = const.tile([128, 1], F32)
            nc.vector.memset(ones_f, 1.0)
            ones_b = const.tile([128, 1], BF16)
            nc.vector.tensor_copy(out=ones_b, in_=ones_f)

            # Preamble DMA order is latency-critical: qt0 gates the first
            # LDWEIGHTS and xt pair 0 the first matmul, so qt0 leads the
            # scalar DGE ring while xt00 leads the sync ring; everything
            # else (qt1, qw, remaining xt pairs) queues behind them. All
            # XT loads are issued upfront with per-pair semaphores so
            # steady-state MM1 never waits on the ring (~32KB/partition).
            def xt_load(b, jp):
                t = xtp.tile([D, 2, CB], F16, tag=f"xt{b}_{jp}")
                nc.sync.dma_start(
                    out=t,
                    in_=XT[b][:, bass.ts(jp, 2 * CB)].rearrange(
                        "d (u c) -> d u c", u=2
                    ),
                )
                return t

            qt_sb = []
            qt0 = const.tile([D, Q_LEN], F16, name="qt0")
            # per-chunk loads: the first LDWEIGHTS gates on a 32KB chunk
            for k in range(NQ):
                nc.scalar.dma_start(
                    out=qt0[:, bass.ts(k, 128)], in_=QT[0][:, bass.ts(k, 128)]
                )
            qt_sb.append(qt0)
            xt_sb = {}
            xt_sb[(0, 0)] = xt_load(0, 0)
            qt1 = const.tile([D, Q_LEN], F16, name="qt1")
            nc.scalar.dma_start(out=qt1, in_=QT[1])
            qt_sb.append(qt1)
            qw_sb = []
            for b in range(BPC):
                qw = const.tile([128, NQ, D], BF16, name=f"qw{b}")
                nc.scalar.dma_start(out=qw, in_=QW[b])
                qw_sb.append(qw)
            for b in range(BPC):
                for jp in range(NBLK // 2):
                    if (b, jp) not in xt_sb:
                        xt_sb[(b, jp)] = xt_load(b, jp)

            # Three-stage software pipeline over the flat block sequence:
            #   stage1(j):   XT DMA (per pair), MM1, exp, partial adds
            #   stage2(j-1): MM2, ones-matmul (s), reciprocal, broadcast
            #   stage3(j-2): normalize-mul, OT store
            # Every tensor op's operands are ready a full block before it
            # issues, so the in-order tensor queue never stalls on the
            # exp -> adds -> s chain of the same block.
            blocks = []
            for b_rep in range(repeat * BPC):
                for jp in range(NBLK // 2):
                    for u in range(2):
                        blocks.append((b_rep, jp, u))
            st1 = {}  # j -> (e_pairs, part, o_pair, blk)
            st2 = {}  # j -> (o_ps, r_bc, o_pair, blk)

            def stage1(j):
                b_rep, jp, u = blocks[j]
                b = b_rep % BPC
                if u == 0:
                    o_pair = osbp.tile([D, 2, CB], BF16, tag="osb")
                    st1["o_pair"] = o_pair
                xt_blk = xt_sb[(b, jp)][:, u, :]

                # MM1 into paired PSUM tiles; one exp per pair (halves the
                # ACTIVATE fixed overhead), E in bf16
                e_pairs = []
                for h in range(NQ // 2):
                    st = ps_st.tile([128, 2, CB], F32, tag="st")
                    for i in range(2):
                        k = 2 * h + i
                        nc.tensor.matmul(
                            out=st[:, i, :],
                            lhsT=qt_sb[b][:, bass.ts(k, 128)],
                            rhs=xt_blk,
                            start=True,
                            stop=True,
                        )
                    e = ep.tile([128, 2, CB], BF16, tag="e")
                    nc.scalar.activation(
                        out=e, in_=st, func=mybir.ActivationFunctionType.Exp
                    )
                    e_pairs.append(e)

                # partial sums over q chunks (bf16, 2x DVE rate): one
                # full-pair [128,1024] add, then a 512-wide fold
                with nc.allow_low_precision(
                    reason="bf16 partial softmax sums; denominator only "
                    "needs ~3 digits (gate is 2e-2)"
                ):
                    q0 = paddp.tile([128, 2, CB], BF16, tag="q0")
                    nc.vector.tensor_add(
                        out=q0, in0=e_pairs[0], in1=e_pairs[1]
                    )
                    part = paddp.tile([128, CB], BF16, tag="part")
                    nc.vector.tensor_add(
                        out=part, in0=q0[:, 0, :], in1=q0[:, 1, :]
                    )
                st1[j] = (e_pairs, part, st1["o_pair"], (b_rep, jp, u))

            def s_chain(j):
                # s for block j-2: part is two iterations old, so the
                # ones-matmul (first on this iteration's tensor queue)
                # never waits; recip/broadcast complete with a full
                # iteration of slack before the mul consumes r_bc.
                part = st1[j][1]
                s_ps = ps_s.tile([1, CB], F32)
                nc.tensor.matmul(
                    out=s_ps, lhsT=ones_b, rhs=part, start=True, stop=True
                )
                # single-instruction approximate reciprocal (~18 bits)
                r_sb = rp.tile([1, CB], F32)
                nc.vector.reciprocal_approx_fast(out=r_sb, in_=s_ps)
                # broadcast recip row across partitions on GPSIMD
                r_bc = rbcp.tile([128, CB], F32)
                nc.gpsimd.partition_broadcast(r_bc, r_sb)
                st2[j] = r_bc

            def mm2(j):
                e_pairs, part, o_pair, (b_rep, jp, u) = st1[j]
                b = b_rep % BPC
                # MM2: OT1 = QW^T E (unnormalized c2q@W1, transposed)
                o_ps = ps_o.tile([D, CB], F32)
                for h in range(NQ // 2):
                    for i in range(2):
                        k = 2 * h + i
                        nc.tensor.matmul(
                            out=o_ps,
                            lhsT=qw_sb[b][:, k, :],
                            rhs=e_pairs[h][:, i, :],
                            start=(k == 0),
                            stop=(k == NQ - 1),
                        )
                st1[j] = (e_pairs, part, o_pair, (b_rep, jp, u), o_ps)

            def mul_store(j):
                _, _, o_pair, (b_rep, jp, u), o_ps = st1.pop(j)
                r_bc = st2.pop(j)
                # normalize while moving PSUM->SBUF (bf16 out)
                with nc.allow_low_precision(
                    reason="bf16 OT output; 2e-2 gate, measured ~4e-3"
                ):
                    nc.vector.tensor_mul(out=o_pair[:, u, :], in0=o_ps, in1=r_bc)
                if CS is not None:
                    nc.sync.dma_start(
                        out=CS[b_rep : b_rep + 1, 2 * jp + u : 2 * jp + u + 1],
                        in_=o_pair[0:1, u, 0:1],
                    )
                last_pair = jp == NBLK // 2 - 1 and b_rep == repeat * BPC - 1
                if repeat == 1 or b_rep >= (repeat - 1) * BPC:
                    if last_pair:
                        # final pair: store per block so the last DMA
                        # starts right after its mul instead of waiting
                        # for both muls
                        nc.sync.dma_start(
                            out=OT[b_rep % BPC][
                                :, bass.ts(2 * jp + u, CB)
                            ],
                            in_=o_pair[:, u, :],
                        )
                    elif u == 1:
                        nc.sync.dma_start(
                            out=OT[b_rep % BPC][
                                :, bass.ts(jp, 2 * CB)
                            ].rearrange("d (u c) -> d u c", u=2),
                            in_=o_pair,
                        )

            n = len(blocks)
            for j in range(n + 2):
                if j >= 2:
                    s_chain(j - 2)
                if j < n:
                    stage1(j)
                if j >= 1 and j - 1 < n:
                    mm2(j - 1)
                if j >= 2:
                    mul_store(j - 2)

    nc.compile()
    return nc


def _prep_inputs(x_cont, x_ques, W1):
    bf16 = mybir.dt.np(BF16)
    xt = np.ascontiguousarray(
        x_cont.transpose(0, 2, 1), dtype=np.float16
    )                                                          # [B, D, C] f16
    qt = np.ascontiguousarray(
        x_ques.transpose(0, 2, 1), dtype=np.float16
    )                                                          # [B, D, Q] f16
    qw = np.matmul(x_ques, W1).astype(bf16)                    # [B, Q, D] bf16
    # device-side stationary layout: [B, 128, NQ, D], contiguous DMA lines
    qw = np.ascontiguousarray(
        qw.reshape(B, Q_LEN // 128, 128, D).transpose(0, 2, 1, 3)
    )
    return xt, qt, qw


def kernel(x_cont, x_ques, c_mask, q_mask, W0, W1):
    x_cont = np.ascontiguousarray(x_cont, dtype=np.float32)
    x_ques = np.ascontiguousarray(x_ques, dtype=np.float32)
    W0 = np.ascontiguousarray(W0, dtype=np.float32)
    W1 = np.ascontiguousarray(W1, dtype=np.float32)

    if "nc" not in _CACHE:
        _CACHE["nc"] = _build()
    nc = _CACHE["nc"]

    xt, qt, qw = _prep_inputs(x_cont, x_ques, W1)

    in_maps = []
    for i in range(NCORES):
        sl = slice(i * BPC, (i + 1) * BPC)
        in_maps.append({"XT": xt[sl], "QT": qt[sl], "QW": qw[sl]})

    res = run_bass_kernel_spmd(nc, in_maps, core_ids=list(range(NCORES)))

    out = np.matmul(x_cont, W0)  # [B, C, D] — attention-free term, on host
    for i in range(NCORES):
        ot = np.asarray(res.results[i]["OT"]).astype(np.float32)  # [BPC, D, C]
        out[i * BPC : (i + 1) * BPC] += ot.transpose(0, 2, 1)
    return out


# --- timing helper for test.py (not used by the graded kernel() path) ---
def timed_run(x_cont, x_ques, W0, W1, iters=10):
    """Persistent-jit execution; returns (list of wall times per exec, out).

    Replicates bass2jax.run_bass_via_pjrt but keeps the jitted callable and
    device-resident inputs across iterations so the measured time is
    dispatch + NEFF execution, not retracing/host transfers.
    """
    import time

    import jax
    from jax.sharding import Mesh, PartitionSpec
    from jax.experimental.shard_map import shard_map

    import concourse.mybir as _mybir
    from concourse import bass2jax

    if "nc" not in _CACHE:
        _CACHE["nc"] = _build()
    nc = _CACHE["nc"]
    bass2jax.install_neuronx_cc_hook()

    xt, qt, qw = _prep_inputs(x_cont, x_ques, W1)
    full = {"XT": xt, "QT": qt, "QW": qw}

    partition_name = nc.partition_id_tensor.name if nc.partition_id_tensor else None
    in_names, out_names, out_avals, zero_outs = [], [], [], []
    for alloc in nc.m.functions[0].allocations:
        if not isinstance(alloc, _mybir.MemoryLocationSet):
            continue
        name = alloc.memorylocations[0].name
        if alloc.kind == "ExternalInput":
            if name != partition_name:
                in_names.append(name)
        elif alloc.kind == "ExternalOutput":
            shape = tuple(alloc.tensor_shape)
            dtype = _mybir.dt.np(alloc.dtype)
            out_names.append(name)
            out_avals.append(jax.core.ShapedArray(shape, dtype))
            zero_outs.append(np.zeros(shape, dtype))
    n_params = len(in_names)
    n_outs = len(out_avals)
    all_names = in_names + out_names
    if partition_name is not None:
        all_names = all_names + [partition_name]

    def _body(*args):
        operands = list(args)
        if partition_name is not None:
            operands.append(bass2jax.partition_id_tensor())
        outs = bass2jax._bass_exec_p.bind(
            *operands,
            out_avals=tuple(out_avals),
            in_names=tuple(all_names),
            out_names=tuple(out_names),
            lowering_input_output_aliases=(),
            sim_require_finite=True,
            sim_require_nnan=True,
            nc=nc,
        )
        return tuple(outs)

    devices = jax.devices()[:NCORES]
    mesh = Mesh(np.asarray(devices), ("core",))
    spec = PartitionSpec("core")
    donate = tuple(range(n_params, n_params + n_outs))
    sharded = jax.jit(
        shard_map(
            _body,
            mesh=mesh,
            in_specs=(spec,) * (n_params + n_outs),
            out_specs=(spec,) * n_outs,
            check_rep=False,
        ),
        donate_argnums=donate,
        keep_unused=True,
    )

    sharding = jax.sharding.NamedSharding(mesh, spec)
    concat_in = [
        jax.device_put(np.ascontiguousarray(full[name]), sharding)
        for name in in_names
    ]

    def fresh_zeros():
        return [
            jax.device_put(
                np.zeros((NCORES * z.shape[0], *z.shape[1:]), z.dtype), sharding
            )
            for z in zero_outs
        ]

    out_arrs = sharded(*concat_in, *fresh_zeros())
    jax.block_until_ready(out_arrs)

    zsets = [fresh_zeros() for _ in range(iters)]
    times = []
    for zs in zsets:
        t0 = time.perf_counter()
        out_arrs = sharded(*concat_in, *zs)
        jax.block_until_ready(out_arrs)
        times.append(time.perf_counter() - t0)
    return times, out_arrs
